# revision 20
# baseline (speedup 1.0000x reference)
"""2-layer GCN encoder on 8 Trainium2 NeuronCores (Bass/Tile).

Strategy (per sharding hint): nodes sharded across 8 cores; edges partitioned
by destination node; per-layer feature tables AllGathered; scatter-add done
locally per dst shard via one-hot matmuls on the tensor engine.

Math: GCNConv's symmetric norm factorizes: norm(e) = dinv[src]*dinv[dst].
With g = dinv * (x @ W), the aggregation is out[d] = dinv[d]*(sum_{e:dst=d}
g[src(e)] + g[d]) + b  (the +g[d] term is the self-loop).

V2 layout/overlap improvements over the first working version:
 - all index metadata preloaded to SBUF once (no per-window idx DMAs)
 - x preloaded via one large partition-major DMA (host supplies xT layout)
 - self rows served from SBUF staging tiles (no DRAM round trip)
 - table-shard DRAM writes batched into 4 chunked DMAs per layer
 - one dma_gather call per (window, half) (ring allows ~2048 descs)
 - layer-2 AllGather chunked into 4 pieces overlapped with layer-1 compute
 - edges sorted by src row within each (window, half) group for DMA locality
"""

import math
import sys

for _p in ("/opt/trn_rl_repo", "/opt/trn_rl_repo/concourse"):
    if _p not in sys.path:
        sys.path.insert(0, _p)

import ml_dtypes
import numpy as np

import concourse.bacc as bacc
import concourse.bass as bass
import concourse.mybir as mybir
import concourse.tile as tile
from concourse.bass_utils import run_bass_kernel_spmd
from concourse.library_config import mlp as _mlp_lib

F32 = mybir.dt.float32
F32R = mybir.dt.float32r
BF16 = mybir.dt.bfloat16
I16 = mybir.dt.int16
AF = mybir.ActivationFunctionType
ALU = mybir.AluOpType

C = 8          # cores
P = 128        # partitions / window size

TABLE_DT = BF16


def _ceil(a, b):
    return (a + b - 1) // b


N_CHUNKS = 4


def window_chunks(NT):
    bnds = [round(i * NT / N_CHUNKS) for i in range(N_CHUNKS + 1)]
    return bnds, [(bnds[i], bnds[i + 1]) for i in range(N_CHUNKS)]


class Plan:
    def __init__(self, N, E, DIN, DH, DOUT, KA, KB):
        self.N, self.E, self.DIN, self.DH, self.DOUT = N, E, DIN, DH, DOUT
        self.NLOC = N // C
        self.NT = _ceil(self.NLOC, P)          # windows (node tiles) per core
        self.NLOC_PAD = self.NT * P
        self.TBL = C * self.NLOC_PAD           # gather-table rows
        # chunk-major table layout: [chunk][core][chunk windows x 128]
        # (chunked AllGather outputs must be contiguous)
        bnds, _ = window_chunks(self.NT)
        self.HALF = C * P * bnds[N_CHUNKS // 2]
        assert self.HALF <= 32768, self.HALF
        assert self.TBL - self.HALF <= 32768, self.TBL - self.HALF
        self.KA, self.KB = KA, KB              # chunks per (window, half)
        self.KW = KA + KB


def preprocess(x, edge_index, W1, b1, W2, b2):
    """Host-side sharding: integer index metadata only."""
    N, DIN = x.shape
    E = edge_index.shape[1]
    DH = W1.shape[1]
    DOUT = W2.shape[1]

    src = np.asarray(edge_index[0], dtype=np.int64)
    dst = np.asarray(edge_index[1], dtype=np.int64)

    NLOC = N // C
    NT = _ceil(NLOC, P)
    NLOC_PAD = NT * P
    TBL = C * NLOC_PAD
    _b, _ = window_chunks(NT)
    HALF = C * P * _b[N_CHUNKS // 2]

    deg = (np.bincount(dst, minlength=N) + 1).astype(np.float32)

    # chunk-major table row: base[i] + c*rows_i + (w - lo_i)*128 + p
    bnds, chs = window_chunks(NT)
    ids = np.arange(N, dtype=np.int64)
    id_c = ids // NLOC
    id_loc = ids % NLOC
    id_w = id_loc // P
    id_p = id_loc % P
    chunk_of_w = np.zeros(NT, dtype=np.int64)
    for i, (lo, hi) in enumerate(chs):
        chunk_of_w[lo:hi] = i
    base = np.array([C * P * b for b in bnds], dtype=np.int64)
    ch = chunk_of_w[id_w]
    rows_per_core = np.array([P * (hi - lo) for lo, hi in chs], dtype=np.int64)
    lo_of = np.array([lo for lo, hi in chs], dtype=np.int64)
    maprow = (base[ch] + id_c * rows_per_core[ch]
              + (id_w - lo_of[ch]) * P + id_p)

    core_of = dst // NLOC
    dst_loc = dst - core_of * NLOC
    w_of = dst_loc // P
    rel = (dst_loc % P).astype(np.int16)
    srcrow = maprow[src]
    half = srcrow >= HALF
    idx16 = (srcrow - half * HALF).astype(np.int16)

    # group edges by (core, window, half); sort by src row within group
    gid = (core_of * NT + w_of) * 2 + half
    order = np.argsort(gid * (1 << 16) + srcrow, kind="stable")
    gid_s = gid[order]
    idx16_s = idx16[order]
    rel_s = rel[order]
    n_groups = C * NT * 2
    counts = np.bincount(gid_s, minlength=n_groups)
    starts = np.concatenate([[0], np.cumsum(counts)[:-1]])

    cA = counts.reshape(C, NT, 2)[:, :, 0]
    cB = counts.reshape(C, NT, 2)[:, :, 1]
    KA = max(1, _ceil(int(cA.max()), P))
    KB = max(1, _ceil(int(cB.max()), P))
    plan = Plan(N, E, DIN, DH, DOUT, KA, KB)
    KW = plan.KW

    def wrap_idx(a):
        # [n] -> [128, n//16] wrapped in 16 partitions, replicated x8
        n = a.shape[0]
        w = a.reshape(n // 16, 16).T  # [16, n//16]
        return np.tile(w, (8, 1))

    in_maps = []
    for c in range(C):
        lo, hi = c * NLOC, (c + 1) * NLOC
        # partition-major x: xT[p, w*DIN:(w+1)*DIN] = x[lo + w*128 + p]
        xsh = np.zeros((NLOC_PAD, DIN), np.float32)
        xsh[:NLOC] = x[lo:hi]
        xT = np.ascontiguousarray(
            xsh.reshape(NT, P, DIN).transpose(1, 0, 2).reshape(P, NT * DIN))
        xT = xT.astype(ml_dtypes.bfloat16)

        degf = np.ones((NT * P,), np.float32)
        degf[:NLOC] = deg[lo:hi]
        degf = degf.reshape(NT, P).T.copy()  # [P, NT]

        idxA = np.zeros((NT, plan.KA * P), np.int16)
        idxB = np.zeros((NT, plan.KB * P), np.int16)
        dstrel = np.full((NT, KW * P), -1, np.int16)
        for w in range(NT):
            gA = (c * NT + w) * 2
            nA = counts[gA]
            sA = starts[gA]
            idxA[w, :nA] = idx16_s[sA:sA + nA]
            dstrel[w, :nA] = rel_s[sA:sA + nA]
            gB = gA + 1
            nB = counts[gB]
            sB = starts[gB]
            idxB[w, :nB] = idx16_s[sB:sB + nB]
            dstrel[w, plan.KA * P:plan.KA * P + nB] = rel_s[sB:sB + nB]

        idxA_in = np.concatenate([wrap_idx(idxA[w]) for w in range(NT)], axis=1)
        idxB_in = np.concatenate([wrap_idx(idxB[w]) for w in range(NT)], axis=1)
        dr = dstrel.reshape(NT * KW, P).T.copy()

        in_maps.append({
            "xT": xT,
            "W1": np.asarray(W1, np.float32),
            "W2": np.asarray(W2, np.float32),
            "b1r": np.tile(np.asarray(b1, np.float32)[None, :], (P, 1)),
            "b2r": np.tile(np.asarray(b2, np.float32)[None, :], (P, 1)),
            "degf": degf,
            "idxA": idxA_in,
            "idxB": idxB_in,
            "dstrel": dr,
        })
    return plan, in_maps


def build(plan: Plan):
    DIN, DH, DOUT = plan.DIN, plan.DH, plan.DOUT
    NT, KA, KB, KW = plan.NT, plan.KA, plan.KB, plan.KW
    NLOC, NLOC_PAD, TBL, HALF = plan.NLOC, plan.NLOC_PAD, plan.TBL, plan.HALF

    # window chunks for batched table writes / chunked AllGathers
    bnds, chunks = window_chunks(NT)

    nc = bacc.Bacc("TRN2", target_bir_lowering=False, debug=False, num_devices=C,
                   dynamic_dma_scratch_size=65536, num_swdge_queues=4)

    xT = nc.dram_tensor("xT", [P, NT * DIN], BF16, kind="ExternalInput")
    W1 = nc.dram_tensor("W1", [DIN, DH], F32R, kind="ExternalInput")
    W2 = nc.dram_tensor("W2", [DH, DOUT], F32R, kind="ExternalInput")
    b1r = nc.dram_tensor("b1r", [P, DH], F32, kind="ExternalInput")
    b2r = nc.dram_tensor("b2r", [P, DOUT], F32, kind="ExternalInput")
    degf = nc.dram_tensor("degf", [P, NT], F32, kind="ExternalInput")
    idxA = nc.dram_tensor("idxA", [P, NT * KA * 8], I16, kind="ExternalInput")
    idxB = nc.dram_tensor("idxB", [P, NT * KB * 8], I16, kind="ExternalInput")
    dstrel = nc.dram_tensor("dstrel", [P, NT * KW], I16, kind="ExternalInput")
    out = nc.dram_tensor("out", [NLOC, DOUT], F32, kind="ExternalOutput")

    g1_in = nc.dram_tensor("g1_in", [NLOC_PAD, DIN], TABLE_DT)
    g1_tbl = nc.dram_tensor("g1_tbl", [TBL, DIN], TABLE_DT, addr_space="Shared")
    g2_in = nc.dram_tensor("g2_in", [NLOC_PAD, DOUT], TABLE_DT)
    g2_tbl = nc.dram_tensor("g2_tbl", [TBL, DOUT], TABLE_DT, addr_space="Shared")

    with tile.TileContext(nc) as tc:
        with tc.tile_pool(name="const", bufs=1) as cpool, \
             tc.tile_pool(name="pay", bufs=4) as paypool, \
             tc.tile_pool(name="sbuild", bufs=2) as spool, \
             tc.tile_pool(name="epi", bufs=2) as epool, \
             tc.tile_pool(name="psum", bufs=2, space="PSUM") as pspool, \
             tc.tile_pool(name="psumT", bufs=2, space="PSUM") as ptpool:

            nc.gpsimd.load_library(_mlp_lib)

            # ---- constants / preloads ----
            W1_sb = cpool.tile([P, DH], F32R, tag="W1")
            nc.sync.dma_start(W1_sb[:, :], W1[:, :])
            W2_sb = [cpool.tile([P, DOUT], F32R, tag=f"W2_{k}", name=f"W2_{k}")
                     for k in range(DH // P)]
            for k in range(DH // P):
                nc.sync.dma_start(W2_sb[k][:, :], W2[k * P:(k + 1) * P, :])
            b1_sb = cpool.tile([P, DH], F32, tag="b1")
            nc.sync.dma_start(b1_sb[:, :], b1r[:, :])
            b2_sb = cpool.tile([P, DOUT], F32, tag="b2")
            nc.sync.dma_start(b2_sb[:, :], b2r[:, :])

            deg_sb = cpool.tile([P, NT], F32, tag="deg")
            nc.sync.dma_start(deg_sb[:, :], degf[:, :])
            sq_sb = cpool.tile([P, NT], F32, tag="sqdeg")
            nc.scalar.activation(sq_sb[:, :], deg_sb[:, :], AF.Sqrt)
            dinv_sb = cpool.tile([P, NT], F32, tag="dinv")
            nc.vector.reciprocal(dinv_sb[:, :], sq_sb[:, :])

            dstrel_sb = cpool.tile([P, NT * KW], I16, tag="dstrel")
            nc.sync.dma_start(dstrel_sb[:, :], dstrel[:, :])
            idxA_sb = cpool.tile([P, NT * KA * 8], I16, tag="idxA")
            nc.sync.dma_start(idxA_sb[:, :], idxA[:, :])
            idxB_sb = cpool.tile([P, NT * KB * 8], I16, tag="idxB")
            nc.sync.dma_start(idxB_sb[:, :], idxB[:, :])

            xT_sb = cpool.tile([P, NT, DIN], BF16, tag="xT")
            nc.sync.dma_start(xT_sb[:, :, :],
                              xT.ap().rearrange("p (w d) -> p w d", w=NT))

            # iota along free (m = 0..127 repeated KW times), int16
            iota_sb = cpool.tile([P, KW, P], I16, tag="iota")
            nc.gpsimd.iota(iota_sb[:, :, :], pattern=[[0, KW], [1, P]],
                           base=0, channel_multiplier=0)
            iota_p = cpool.tile([P, P], I16, tag="iota_p")
            nc.gpsimd.iota(iota_p[:, :], pattern=[[0, P]], base=0,
                           channel_multiplier=1)
            ident = cpool.tile([P, P], TABLE_DT, tag="ident")
            nc.vector.tensor_tensor(ident[:, :], iota_sb[:, 0, :], iota_p[:, :],
                                    ALU.is_equal)
            identf = cpool.tile([P, P], F32, tag="identf")
            nc.vector.tensor_tensor(identf[:, :], iota_sb[:, 0, :], iota_p[:, :],
                                    ALU.is_equal)

            # staging tiles for the table shards (self rows + batched writes)
            g1st = [cpool.tile([P, (hi - lo) * DIN], TABLE_DT, tag=f"g1st{i}",
                               name=f"g1st{i}")
                    for i, (lo, hi) in enumerate(chunks)]
            g2st = [cpool.tile([P, (hi - lo) * DOUT], TABLE_DT, tag=f"g2st{i}",
                               name=f"g2st{i}")
                    for i, (lo, hi) in enumerate(chunks)]

            def chunk_of(w):
                for i, (lo, hi) in enumerate(chunks):
                    if lo <= w < hi:
                        return i, lo
                raise AssertionError(w)

            # ---- layer-1 producer: g1 = dinv * x (bf16) ----
            for i, (lo, hi) in enumerate(chunks):
                for w in range(lo, hi):
                    nc.scalar.activation(
                        g1st[i][:, (w - lo) * DIN:(w - lo + 1) * DIN],
                        xT_sb[:, w, :], AF.Copy, scale=dinv_sb[:, w:w + 1])
                nc.sync.dma_start(
                    g1_in.ap()[lo * P:hi * P, :].rearrange(
                        "(s p) d -> p s d", p=P),
                    g1st[i][:, :].rearrange("p (s d) -> p s d", s=hi - lo))

            # AllGather layer-1 table, chunked (chunk-major table layout)
            for i, (lo, hi) in enumerate(chunks):
                nc.gpsimd.collective_compute(
                    "AllGather", ALU.bypass,
                    replica_groups=[list(range(C))],
                    ins=[g1_in.ap()[lo * P:hi * P, :].opt()],
                    outs=[g1_tbl.ap()[C * P * lo:C * P * hi, :].opt()])

            def aggregate(w, tbl, selfg, dim):
                """psum tile with sum_{e:dst in window} g[src] + g[dst]."""
                payA = paypool.tile([P, KA, dim], TABLE_DT, tag=f"payA{dim}",
                                    name=f"payA{dim}")
                payB = paypool.tile([P, KB, dim], TABLE_DT, tag=f"payB{dim}",
                                    name=f"payB{dim}")
                # <=896 descriptors per gather call (ucode ring limit);
                # split each half into near-equal calls for queue balance
                MAXC = 7
                q = 5 * w

                def split(k):
                    n = _ceil(k, MAXC)
                    base, rem = k // n, k % n
                    return [base + (j < rem) for j in range(n)]

                s = 0
                for cn in split(KA):
                    nc.gpsimd.dma_gather(
                        payA[:, s:s + cn, :], tbl[0:HALF, :],
                        idxA_sb[:, (w * KA + s) * 8:(w * KA + s + cn) * 8],
                        cn * P, cn * P, dim, queue_num=q % 4)
                    q += 1
                    s += cn
                s = 0
                for cn in split(KB):
                    nc.gpsimd.dma_gather(
                        payB[:, s:s + cn, :], tbl[HALF:2 * HALF, :],
                        idxB_sb[:, (w * KB + s) * 8:(w * KB + s + cn) * 8],
                        cn * P, cn * P, dim, queue_num=q % 4)
                    q += 1
                    s += cn
                S = spool.tile([P, KW, P], TABLE_DT, tag="S")
                drel = dstrel_sb[:, w * KW:(w + 1) * KW]
                nc.vector.tensor_tensor(
                    S[:, :, :],
                    drel.unsqueeze(-1).broadcast_to((P, KW, P)),
                    iota_sb[:, :, :], ALU.is_equal)
                ps = pspool.tile([P, dim], F32, tag="aggps")
                nc.tensor.matmul(ps[:, :], ident[:, :],
                                 selfg, start=True, stop=False)
                for k in range(KA):
                    nc.tensor.matmul(ps[:, :], S[:, k, :],
                                     payA[:, k, :],
                                     start=False, stop=False)
                for k in range(KB):
                    nc.tensor.matmul(ps[:, :], S[:, KA + k, :],
                                     payB[:, k, :],
                                     start=False, stop=(k == KB - 1))
                return ps

            # ---- layer 1 consume + layer-2 producer (AG2 chunked+overlapped) ----
            for i, (lo, hi) in enumerate(chunks):
                for w in range(lo, hi):
                    ps = aggregate(w, g1_tbl,
                                   g1st[i][:, (w - lo) * DIN:(w - lo + 1) * DIN],
                                   DIN)
                    aggs = epool.tile([P, DIN], F32, tag="aggs")
                    nc.scalar.activation(aggs[:, :], ps[:, :], AF.Copy,
                                         scale=dinv_sb[:, w:w + 1])
                    pt1 = ptpool.tile([P, P], F32, tag="pt")
                    nc.tensor.transpose(pt1[:, :], aggs[:, :], identf[:, :])
                    aggT = epool.tile([P, DIN], F32R, tag="aggT")
                    nc.vector.tensor_copy(aggT[:, :], pt1[:, :])
                    ps1 = pspool.tile([P, DH], F32, tag="gps")
                    nc.tensor.matmul(ps1[:, :], aggT[:, :], W1_sb[:, :],
                                     start=True, stop=True)
                    v = epool.tile([P, DH], F32, tag="v1")
                    nc.vector.scalar_tensor_tensor(v[:, :], ps1[:, :],
                                                   1.0, b1_sb[:, :],
                                                   ALU.mult, ALU.add)
                    h1 = epool.tile([P, DH], F32, tag="h1")
                    nc.scalar.activation(h1[:, :], v[:, :], AF.Relu)
                    # z2 = h1 @ W2 ; g2 = dinv * z2
                    hT = []
                    for k in range(DH // P):
                        pt = ptpool.tile([P, P], F32, tag="pt")
                        nc.tensor.transpose(pt[:, :], h1[:, k * P:(k + 1) * P],
                                            identf[:, :])
                        hTk = epool.tile([P, P], F32R, tag=f"hT{k}",
                                         name=f"hT{k}")
                        nc.vector.tensor_copy(hTk[:, :], pt[:, :])
                        hT.append(hTk)
                    ps2 = pspool.tile([P, DOUT], F32, tag="z2ps")
                    for k in range(DH // P):
                        nc.tensor.matmul(ps2[:, :], hT[k][:, :],
                                         W2_sb[k][:, :],
                                         start=(k == 0), stop=(k == DH // P - 1))
                    nc.scalar.activation(
                        g2st[i][:, (w - lo) * DOUT:(w - lo + 1) * DOUT],
                        ps2[:, :], AF.Copy, scale=dinv_sb[:, w:w + 1])
                # chunk finished: batched write + chunked AllGather
                nc.sync.dma_start(
                    g2_in.ap()[lo * P:hi * P, :].rearrange(
                        "(s p) d -> p s d", p=P),
                    g2st[i][:, :].rearrange("p (s d) -> p s d", s=hi - lo))
                nc.gpsimd.collective_compute(
                    "AllGather", ALU.bypass,
                    replica_groups=[list(range(C))],
                    ins=[g2_in.ap()[lo * P:hi * P, :].opt()],
                    outs=[g2_tbl.ap()[C * P * lo:C * P * hi, :].opt()])

            # ---- layer 2 consume + normalize ----
            for i, (lo, hi) in enumerate(chunks):
                for w in range(lo, hi):
                    ps = aggregate(w, g2_tbl,
                                   g2st[i][:, (w - lo) * DOUT:(w - lo + 1) * DOUT],
                                   DOUT)
                    v = epool.tile([P, DOUT], F32, tag="v2")
                    nc.vector.scalar_tensor_tensor(v[:, :], ps[:, :],
                                                   dinv_sb[:, w:w + 1], b2_sb[:, :],
                                                   ALU.mult, ALU.add)
                    sq = epool.tile([P, DOUT], F32, tag="sq")
                    ss = epool.tile([P, 1], F32, tag="ss")
                    nc.scalar.activation(sq[:, :], v[:, :], AF.Square,
                                         accum_out=ss[:, :])
                    ssm = epool.tile([P, 1], F32, tag="ssm")
                    nc.vector.tensor_scalar_max(ssm[:, :], ss[:, :], 1e-24)
                    sr = epool.tile([P, 1], F32, tag="sr")
                    nc.scalar.activation(sr[:, :], ssm[:, :], AF.Sqrt)
                    inv = epool.tile([P, 1], F32, tag="inv")
                    nc.vector.reciprocal(inv[:, :], sr[:, :])
                    ot = epool.tile([P, DOUT], F32, tag="ot")
                    nc.scalar.activation(ot[:, :], v[:, :], AF.Copy,
                                         scale=inv[:, 0:1])
                    rows = min(P, NLOC - w * P)
                    nc.sync.dma_start(out[w * P:w * P + rows, :], ot[:rows, :])

    nc.compile()
    return nc


_CACHE = {}


def kernel(x, edge_index, W1, b1, W2, b2, **_ignored):
    x = np.asarray(x)
    plan, in_maps = preprocess(x, edge_index, W1, b1, W2, b2)
    key = (x.shape, plan.KA, plan.KB)
    if key not in _CACHE:
        _CACHE[key] = build(plan)
    nc = _CACHE[key]
    res = run_bass_kernel_spmd(nc, in_maps, core_ids=list(range(C)))
    return np.concatenate([res.results[c]["out"] for c in range(C)], axis=0)
